# revision 1
# baseline (speedup 1.0000x reference)
"""Trainium2 Bass kernel for the windowed bidirectional LSTM encoder.

Semantics (derived from the reference): each direction is a plain LSTM cell
chain over a token stream of length 2S-1 (windows overlap, so tokens repeat:
fwd stream = x0,x1,x1,x2,x2,...,x511,x511; bwd stream = x1,x0,x2,x1,...,x511).
The output is the per-feature running max of all 2S-1 hidden states of each
direction, concatenated: emb = [max_t h_f(t) | max_t h_b(t)]  -> (B, 2H).

Distribution: 8 cores, each owns a batch group of 8 rows and runs BOTH
directions (their activation chains hide under each other's matmul phase).

Per-core kernel:
  phase 1: P[d, g, t, :] = X @ Wih_d^T + b_d  for all 512 tokens
           (weights-stationary matmuls, bias folded in the PSUM->SBUF copy)
  phase 2: 1023-step recurrence per direction with Whh stationary (bf16,
           fast-weight-load), gates land in PSUM as (gate-dim x batch),
           LSTM pointwise chain on DVE+ACT, running max of h.

All recurring data is bf16 except c / gates / hmax which stay fp32.
"""

import numpy as np
import ml_dtypes

import concourse.bass as bass
import concourse.mybir as mybir
from concourse import bacc
from concourse.tile import TileContext
from concourse.bass_utils import run_bass_kernel_spmd

F32 = mybir.dt.float32
BF16 = mybir.dt.bfloat16
FP8 = mybir.dt.float8e4
AF = mybir.ActivationFunctionType
ALU = mybir.AluOpType

S = 512
B = 64
E = 256
H = 256
NCORES = 8
BC = B // NCORES          # batch rows per core = 8
NT = 2 * S - 1            # steps per direction = 1023
KT = 2                    # k-tiles (contraction 256 = 2x128)
GT = 8                    # gate tiles (4H = 1024 = 8x128)
TOKCOLS = S * BC          # 4096 moving columns per k-tile in phase 1
CHUNK = 512               # moving cols per phase-1 matmul
NCHUNK = TOKCOLS // CHUNK

# blob column layout (all bf16, 128 partitions):
#  [ X (2*S*BC) | whh_f (2048) | wih_f (2048) | whh_b (2048) | wih_b (2048)
#    | bias_f (8) | bias_b (8) ]
def _blob_layout(s):
    tokcols = s * BC
    xcols = KT * tokcols
    wih_off = [xcols, xcols + 2048]
    bias_off = [xcols + 4096, xcols + 4096 + GT]
    ncols = xcols + 4096 + 2 * GT
    return tokcols, xcols, wih_off, bias_off, ncols

# PSUM gate-tile order: [g g | i i | f f | o o]  (PyTorch order is i,f,g,o)
# rows of the 4H dim, in units of 128: old blocks i:0,1 f:2,3 g:4,5 o:6,7
GATE_ROW_PERM = [4, 5, 0, 1, 2, 3, 6, 7]


def _fwd_tok(t):
    return (t + 1) // 2


def _bwd_tok(t):
    if t == 2 * S - 2:
        return S - 1
    return t // 2 + 1 if t % 2 == 0 else (t - 1) // 2


def _build_program(s=S):
    nt = 2 * s - 1
    tokcols, xcols, wih_off, bias_off, ncols = _blob_layout(s)
    nchunk = tokcols // CHUNK

    nc = bacc.Bacc(None, target_bir_lowering=False)
    blob = nc.dram_tensor("blob", [128, ncols], BF16, kind="ExternalInput")
    whh8 = nc.dram_tensor("whh8", [128, 2 * KT * GT * 128], FP8, kind="ExternalInput")
    out = nc.dram_tensor("out", [128, 2 * 2 * BC], F32, kind="ExternalOutput")

    with TileContext(nc) as tc:
        with (
            tc.tile_pool(name="const", bufs=1) as const_pool,
            tc.tile_pool(name="pbuf", bufs=1) as p_pool,
            tc.tile_pool(name="work", bufs=3) as work,
            tc.tile_pool(name="state", bufs=2) as state,
            tc.tile_pool(name="acc", bufs=1) as acc,
            tc.tile_pool(name="ppsum", bufs=2, space="PSUM") as ppsum,
            tc.tile_pool(name="rpsum", bufs=1, space="PSUM") as rpsum,
        ):
            blob_sb = const_pool.tile([128, ncols], BF16)
            nc.sync.dma_start(blob_sb[:], blob[:])
            whh_sb = const_pool.tile([128, 2 * KT * GT * 128], FP8)
            nc.sync.dma_start(whh_sb[:], whh8[:])

            # P storage: (128, dir, gate-tile, token, batch) bf16
            p_sb = p_pool.tile([128, 2 * GT * tokcols], BF16)
            p_view = p_sb[:].rearrange(
                "p (d g t b) -> p d g t b", d=2, g=GT, t=s, b=BC
            )

            x_view = blob_sb[:, 0:xcols].rearrange(
                "p (k n) -> p k n", k=KT
            )

            def whh_ap(d, k, g):
                off = (d * KT * GT + k * GT + g) * 128
                return whh_sb[:, off:off + 128]

            def wih_ap(d, k, g):
                off = wih_off[d] + (k * GT + g) * 128
                return blob_sb[:, off:off + 128]

            # biases must be fp32 for tensor_scalar: upconvert once
            bias_f32 = const_pool.tile([128, 2 * GT], F32)
            nc.vector.tensor_copy(
                bias_f32[:], blob_sb[:, bias_off[0]:bias_off[0] + 2 * GT]
            )
            # dummy DVE read so the bias dependency is already observed by the
            # DVE vector clock before the first PSUM->SBUF tensor_scalar
            # (walrus allows only ONE sync-wait on a TensorScalar instruction)
            bias_probe = const_pool.tile([128, 1], F32)
            nc.vector.tensor_copy(bias_probe[:], bias_f32[:, 0:1])

            def bias_ap(d, g):
                off = d * GT + g
                return bias_f32[:, off:off + 1]

            # ---------------- phase 1: input projections ----------------
            for d in range(2):
                for g in range(GT):
                    for chk in range(nchunk):
                        ps = ppsum.tile([128, CHUNK], F32, tag="pp")
                        cols = slice(chk * CHUNK, (chk + 1) * CHUNK)
                        for k in range(KT):
                            nc.tensor.matmul(
                                ps[:],
                                wih_ap(d, k, g),
                                x_view[:, k, cols],
                                start=(k == 0),
                                stop=(k == KT - 1),
                            )
                        # bias-folding copy PSUM -> SBUF (bf16)
                        toks = slice(chk * (CHUNK // BC), (chk + 1) * (CHUNK // BC))
                        nc.vector.tensor_scalar(
                            p_view[:, d, g, toks, :],
                            ps[:],
                            bias_ap(d, g),
                            None,
                            ALU.add,
                        )

            # ---------------- phase 2: recurrence ----------------
            # persistent per-direction state
            psum_z = [rpsum.tile([128, GT * BC], F32, tag=f"z{d}", name=f"psum_z{d}") for d in range(2)]
            hmax = [acc.tile([128, 2 * BC], F32, tag=f"hmax{d}", name=f"hmax{d}") for d in range(2)]

            h_cur = [None, None]
            c_cur = [None, None]
            tok_of = [_fwd_tok, lambda t: _bwd_tok_s(t, s)]

            def chain(d, t, z_src):
                """Pointwise LSTM chain from gate pre-activations [g,i,f,o].

                g-gate weights are pre-scaled x2 on the host, so ONE sigmoid
                covers all gates: tanh(zg) = 2*sigmoid(2*zg) - 1.
                ACT outputs share ONE pooled tile (sall) that DVE pre-touches
                so slot-release waits land on the DVE semaphore (walrus allows
                one sync-wait per compute instruction; extra waits cost an
                EventSemaphore instruction).
                Layout: [s_g 2B | s_i 2B | s_f 2B | s_o 2B | th_c 2B].
                """
                w2 = 2 * BC
                sall = work.tile([128, 5 * w2], F32, tag=f"sall{d}", name=f"sall{d}_{t}")
                nc.vector.tensor_copy(sall[:, 0:1], bias_probe[:])
                s_g = sall[:, 0:w2]
                s_i = sall[:, w2:2 * w2]
                s_f = sall[:, 2 * w2:3 * w2]
                s_o = sall[:, 3 * w2:4 * w2]
                th_c = sall[:, 4 * w2:5 * w2]
                nc.scalar.activation(sall[:, 0:4 * w2], z_src, AF.Sigmoid)
                # tanh(zg) = 2*sigmoid(2 zg) - 1, affine done on ACT for free
                tg = work.tile([128, w2], F32, tag="tg", name=f"tg{d}_{t}")
                nc.scalar.activation(tg[:], s_g, AF.Copy, bias=-1.0, scale=2.0)
                c_new = state.tile([128, w2], F32, tag=f"c{d}", name=f"c{d}_{t}")
                if c_cur[d] is None:
                    nc.vector.tensor_mul(c_new[:], s_i, tg[:])
                else:
                    m1 = work.tile([128, w2], F32, tag="m1", name=f"m1_{d}_{t}")
                    nc.vector.tensor_mul(m1[:], s_i, tg[:])
                    m2 = work.tile([128, w2], F32, tag="m2", name=f"m2_{d}_{t}")
                    nc.vector.tensor_mul(m2[:], s_f, c_cur[d][:])
                    nc.vector.tensor_add(c_new[:], m1[:], m2[:])
                nc.scalar.activation(th_c, c_new[:], AF.Tanh)
                h_new = state.tile([128, w2], FP8, tag=f"h{d}", name=f"h{d}_{t}")
                nc.vector.tensor_mul(h_new[:], s_o, th_c)
                # precise running max on the idle GPSIMD engine
                hp = work.tile([128, w2], F32, tag=f"hp{d}", name=f"hp{d}_{t}")
                nc.vector.tensor_mul(hp[:], s_o, th_c)
                if t == 0:
                    nc.vector.tensor_copy(hmax[d][:], hp[:])
                else:
                    nc.vector.tensor_max(hmax[d][:], hmax[d][:], hp[:])
                return h_new, c_new

            # step 0 for both dirs: z = P[tok0] directly (h0 = c0 = 0)
            for d in range(2):
                t0 = tok_of[d](0)
                h_cur[d], c_cur[d] = chain(d, 0, p_view[:, d, :, t0, :])

            for t in range(1, nt):
                for d in range(2):
                    tok = tok_of[d](t)
                    h = h_cur[d]
                    ps = psum_z[d]
                    for g in range(GT):
                        col = g * BC
                        for k in range(KT):
                            nc.tensor.matmul(
                                ps[:, col:col + BC],
                                whh_ap(d, k, g),
                                h[:, k * BC:(k + 1) * BC],
                                start=(k == 0),
                                stop=(k == KT - 1),
                            )
                    z = work.tile([128, GT * BC], F32, tag="z", name=f"z{d}_{t}")
                    nc.vector.tensor_add(z[:], ps[:], p_view[:, d, :, tok, :])
                    h_cur[d], c_cur[d] = chain(d, t, z[:])

            for d in range(2):
                nc.sync.dma_start(out[:, d * 2 * BC:(d + 1) * 2 * BC], hmax[d][:])

    nc.compile()
    return nc


def _bwd_tok_s(t, s):
    if t == 2 * s - 2:
        return s - 1
    return t // 2 + 1 if t % 2 == 0 else (t - 1) // 2


def _pack_blob(X, weights, s=S):
    """Build per-core (128, ncols) bf16 blob + shared (128, 8192) fp8 whh.

    g-gate rows (permuted blocks 0,1) are pre-scaled x2 so the kernel can
    evaluate tanh(zg) = 2*sigmoid(2*zg) - 1 with a single sigmoid call.
    """
    tokcols, xcols, wih_off, bias_off, ncols = _blob_layout(s)
    bf = ml_dtypes.bfloat16

    perm = np.concatenate([np.arange(r * 128, (r + 1) * 128) for r in GATE_ROW_PERM])

    def lhsT_img(W, dtype):
        img = np.empty((128, KT * GT * 128), np.float32)
        for k in range(KT):
            for g in range(GT):
                blockT = W[g * 128:(g + 1) * 128, k * 128:(k + 1) * 128].T
                img[:, (k * GT + g) * 128:(k * GT + g + 1) * 128] = blockT
        return img.astype(dtype)

    wimg = {}
    whh8 = np.empty((128, 2 * KT * GT * 128), ml_dtypes.float8_e4m3)
    for d, nm in enumerate("fb"):
        wih_p = weights[f"wih_{nm}"][perm].copy()
        whh_p = weights[f"whh_{nm}"][perm].copy()
        bias_p = (weights[f"bih_{nm}"] + weights[f"bhh_{nm}"])[perm].copy()
        wih_p[0:256] *= 2.0
        whh_p[0:256] *= 2.0
        bias_p[0:256] *= 2.0
        bimg = np.empty((128, GT), np.float32)
        for g in range(GT):
            bimg[:, g] = bias_p[g * 128:(g + 1) * 128]
        wimg[d] = (lhsT_img(wih_p, bf), bimg)
        whh8[:, d * 2048:(d + 1) * 2048] = lhsT_img(whh_p, ml_dtypes.float8_e4m3)

    Xt = np.ascontiguousarray(np.transpose(X[:s], (2, 0, 1)))  # (E, s, B)
    blobs = []
    for c in range(NCORES):
        img = np.zeros((128, ncols), np.float32)
        xc = Xt[:, :, c * BC:(c + 1) * BC].reshape(KT, 128, tokcols)
        img[:, 0:tokcols] = xc[0]
        img[:, tokcols:2 * tokcols] = xc[1]
        for d in range(2):
            wih_i, b_i = wimg[d]
            img[:, wih_off[d]:wih_off[d] + 2048] = wih_i
            img[:, bias_off[d]:bias_off[d] + GT] = b_i
        blobs.append(img.astype(bf))
    return blobs, whh8


_PROGRAM_CACHE = {}


def _get_program(s=S):
    if s not in _PROGRAM_CACHE:
        _PROGRAM_CACHE[s] = _build_program(s)
    return _PROGRAM_CACHE[s]


def _run(inputs, s=S, trace=False):
    X = np.asarray(inputs["inputs"], np.float32)
    blobs, whh8 = _pack_blob(X, inputs, s=s)
    nc = _get_program(s)
    in_maps = [{"blob": b, "whh8": whh8} for b in blobs]
    res = run_bass_kernel_spmd(nc, in_maps, core_ids=list(range(NCORES)), trace=trace)
    # assemble (B, 2H): out[p, d*2BC + j*BC + b] = h_d[dim 128j+p, batch b]
    emb = np.empty((B, 2 * H), np.float32)
    for c in range(NCORES):
        o = res.results[c]["out"]  # (128, 32)
        for d in range(2):
            for j in range(2):
                blk = o[:, d * 2 * BC + j * BC:d * 2 * BC + (j + 1) * BC]  # (128, BC)
                emb[c * BC:(c + 1) * BC, d * H + j * 128:d * H + (j + 1) * 128] = blk.T
    return emb, res


def kernel(**inputs):
    emb, _ = _run(inputs, s=S, trace=False)
    return emb

